# revision 10
# baseline (speedup 1.0000x reference)
"""DepLabeledGCN Trainium2 kernel — data-parallel variant (no collectives).

Each core processes ITS OWN batch with ALL 48 label matrices:
    s-phase:  sT[kc,l] chunks = per-label masked-adjacency matmuls (fp16,
              masks exact 0/1), label PAIRS fused into N=256 matmuls
    msum:     msg = sum_{l,kc} sT[kc,l] @ W_l^T[kc], 192 accumulating
              matmuls into one PSUM bank per layer
    relu(msg * 1/denom) -> next layer h (4 kc-chunk DVE ops)
then the 2-layer MLP (PE-transpose + packed PSUM) on the same core.

v2 scheduling changes vs baseline:
  - all small inputs packed into ONE dram tensor / one DMA
  - masks for pair 0 + chunked h0 cast first => first matmul ~2.5us earlier
  - sT tile keeps the PSUM layout [q,kc,l,i]; psum->sbuf copy is two
    contiguous halves on different engines (vector/scalar/gpsimd)
  - weight DMAs alternate sync/gpsimd queues (2 DGE streams)
  - relu + h0 casts split into kc chunks to cut layer-boundary latency
  - 16 pair-granular wres loads, 1 MLP-weight load, 1 output DMA
    (fewer queues => shorter semaphore-reset teardown)
"""

import sys

if '/opt/trn_rl_repo' not in sys.path:
    sys.path.insert(0, '/opt/trn_rl_repo')

import numpy as np

B, N, D, L = 8, 128, 512, 48
NCORES = 8
KC = D // 128
NUM_LAYERS = 2
R_RES = 32              # labels kept resident for layer 2
NP = L // 2             # label pairs per layer

# pack column offsets (fp32 units); adjT/labT first so the mask DMA
# chunk can land before the gcn chunk
P_ADJT = 0
P_LABT = 128
P_GCN = 256
P_ADJR = 768
P_B0 = 896
P_B1 = 900
P_TOT = 904
P_SPLIT = 256           # DMA 1: [0, 256) masks;  DMA 2: [256, 904)

_CACHE = {}


def _build_nc():
    import concourse.bass as bass
    import concourse.mybir as mybir
    import concourse.tile as tile
    from concourse import bacc
    from concourse.masks import make_identity

    dt = mybir.dt
    f32 = dt.float32
    f16 = dt.float16
    Alu = mybir.AluOpType

    nc = bacc.Bacc("TRN2", target_bir_lowering=False, debug=False,
                   num_devices=NCORES)

    pack_e = nc.dram_tensor("pack", [N, P_TOT], f32, kind="ExternalInput").ap()
    wT_e = nc.dram_tensor("wT", [128, L, KC, D], f16, kind="ExternalInput").ap()
    mlpw_e = nc.dram_tensor("mlpw", [128, 2, KC, D], f16,
                            kind="ExternalInput").ap()
    out_e = nc.dram_tensor("out", [128, KC, 128], f32,
                           kind="ExternalOutput").ap()

    with tile.TileContext(nc) as tc:
        with (
            tc.tile_pool(name="const", bufs=1) as cpool,
            tc.tile_pool(name="sT", bufs=3) as sT_pool,
            tc.tile_pool(name="wst", bufs=4) as wst_pool,
            tc.tile_pool(name="spsum", bufs=3, space="PSUM") as spsum,
            tc.tile_pool(name="mpsum", bufs=2, space="PSUM") as mpsum,
        ):
            # -------- critical-path input load (two chunks) -----------------
            pack_sb = cpool.tile([128, P_TOT], f32, tag="pack")
            nc.sync.dma_start(pack_sb[:, :P_SPLIT], pack_e[:, :P_SPLIT])
            nc.sync.dma_start(pack_sb[:, P_SPLIT:], pack_e[:, P_SPLIT:])
            gcn_v = pack_sb[:, P_GCN:P_GCN + D]
            adjT_v = pack_sb[:, P_ADJT:P_ADJT + N]
            labT_v = pack_sb[:, P_LABT:P_LABT + N]
            adjR_v = pack_sb[:, P_ADJR:P_ADJR + N]
            b0_v = pack_sb[:, P_B0:P_B0 + KC]
            b1_v = pack_sb[:, P_B1:P_B1 + KC]

            # resident weights, pair-granular, alternating between the sync
            # and scalar HW DMA queues (gpsimd only has a slow SW queue).
            # Scalar issues pair 1 immediately; the rest of its pairs are
            # emitted after the h0 casts so they don't block the first matmul.
            wres = cpool.tile([128, R_RES, KC, D], f16, tag="wres")

            def wres_load(p):
                eng = nc.sync if p % 2 == 0 else nc.scalar
                eng.dma_start(wres[:, 2 * p:2 * p + 2], wT_e[:, 2 * p:2 * p + 2])

            for p in range(R_RES // 2):
                if p % 2 == 0 or p == 1:
                    wres_load(p)

            h = [cpool.tile([128, D], f16, tag=f"h{ly}", name=f"h{ly}")
                 for ly in range(NUM_LAYERS + 1)]

            # -------- masks: maskT[j, l, i] = (labT == l) * adjT ------------
            maskT = cpool.tile([128, L, N], f16, tag="maskT")

            def emit_mask(l):
                nc.vector.scalar_tensor_tensor(
                    out=maskT[:, l, :],
                    in0=labT_v,
                    scalar=float(l),
                    in1=adjT_v,
                    op0=Alu.is_equal,
                    op1=Alu.mult,
                )

            # pair-0 masks first, then h0 chunks (vector/scalar), then more
            emit_mask(0)
            emit_mask(1)
            for kc in range(KC):
                sl = slice(kc * 128, (kc + 1) * 128)
                if kc % 2 == 0:
                    nc.vector.tensor_copy(h[0][:, sl], gcn_v[:, sl])
                else:
                    nc.scalar.copy(h[0][:, sl], gcn_v[:, sl])
            # remaining odd wres pairs, behind the scalar h0 casts
            for p in range(3, R_RES // 2, 2):
                wres_load(p)
            for l in range(2, 12):
                emit_mask(l)

            den = cpool.tile([128, 1], f32, tag="den")
            nc.vector.tensor_reduce(den[:], adjR_v, mybir.AxisListType.X,
                                    Alu.add)
            nc.vector.tensor_scalar_add(den[:], den[:], 1.0)
            recip = cpool.tile([128, 1], f32, tag="recip")
            nc.vector.reciprocal(recip[:], den[:])

            # identity for the MLP transposes (gpsimd, idle at start)
            identity = cpool.tile([128, 128], f16, tag="ident")
            make_identity(nc, identity[:])

            # -------- GCN layers --------------------------------------------
            Act = mybir.ActivationFunctionType

            def emit_s(ly, p):
                """s-phase for label pair p: one N=256 matmul per kc.
                psum AND sbuf tile share layout [q, kc, l, i] so the
                psum->sbuf cast is contiguous; split into two halves on
                vector + scalar (gpsimd has no PSUM access)."""
                ps = spsum.tile([128, KC, 2, 128], f32, tag="spsum",
                                name="spsum")
                for kc in range(KC):
                    nc.tensor.matmul(
                        ps[:, kc, :, :],
                        lhsT=h[ly][:, kc * 128:(kc + 1) * 128],
                        rhs=maskT[:, 2 * p:2 * p + 2, :],
                        start=True, stop=True,
                    )
                sT = sT_pool.tile([128, KC, 2, 128], f16, tag="sT", name="sT")
                nc.vector.tensor_copy(sT[:, 0:2], ps[:, 0:2])
                nc.scalar.copy(sT[:, 2:4], ps[:, 2:4])
                return sT

            def get_w(ly, p):
                """Weight pair p: resident slice or streamed tile."""
                if 2 * p + 1 < R_RES:
                    return wres[:, 2 * p:2 * p + 2]
                w = wst_pool.tile([128, 2, KC, D], f16, tag="wst", name="wst")
                eng = nc.sync if p % 2 == 0 else nc.scalar
                eng.dma_start(w[:], wT_e[:, 2 * p:2 * p + 2])
                return w

            for ly in range(NUM_LAYERS):
                pm = mpsum.tile([128, D], f32, tag="mm", name="mm")
                sT_q = [emit_s(ly, 0), emit_s(ly, 1)]
                for p in range(NP):
                    if ly == 0 and 2 * (p + 6) < L:
                        emit_mask(2 * (p + 6))
                        emit_mask(2 * (p + 6) + 1)
                    if p + 2 < NP:
                        sT_q.append(emit_s(ly, p + 2))
                    w = get_w(ly, p)
                    sT = sT_q[p]
                    for kc in range(KC):
                        for l2 in range(2):
                            i = p * 2 * KC + kc * 2 + l2
                            nc.tensor.matmul(
                                pm[:],
                                lhsT=sT[:, kc, l2, :],
                                rhs=w[:, l2, kc, :],
                                start=(i == 0), stop=(i == L * KC - 1),
                            )
                if ly == 0:
                    # MLP weights: load during layer 2 (slack window)
                    mlpw_sb = cpool.tile([128, 2, KC, D], f16, tag="mlpw")
                    nc.sync.dma_start(mlpw_sb[:], mlpw_e)
                # relu(msg * recip) -> next h (fp16), chunked per kc
                for kc in range(KC):
                    sl = slice(kc * 128, (kc + 1) * 128)
                    if kc % 2 == 0:
                        nc.vector.tensor_scalar(h[ly + 1][:, sl], pm[:, sl],
                                                recip[:], 0.0,
                                                Alu.mult, Alu.max)
                    else:
                        nc.scalar.activation(h[ly + 1][:, sl], pm[:, sl],
                                             Act.Relu, scale=recip[:])

            # -------- MLP ---------------------------------------------------
            w0T_v = mlpw_sb[:, 0]
            w1T_v = mlpw_sb[:, 1]
            h_own = h[NUM_LAYERS]
            hT = cpool.tile([128, KC, 128], f16, tag="hT")
            pt = mpsum.tile([128, KC, 128], f16, tag="mm", name="ptr")
            for kc in range(KC):
                nc.tensor.transpose(pt[:, kc, :],
                                    h_own[:, kc * 128:(kc + 1) * 128],
                                    identity[:])
            nc.vector.tensor_copy(hT[:, 0:2], pt[:, 0:2])
            nc.scalar.copy(hT[:, 2:4], pt[:, 2:4])

            x1T = cpool.tile([128, KC, 128], f16, tag="x1T")
            px1 = mpsum.tile([128, KC, 128], f32, tag="mm", name="px1")
            for blk in range(KC):
                for kc in range(KC):
                    nc.tensor.matmul(
                        px1[:, blk, :],
                        lhsT=w0T_v[:, kc, blk * 128:(blk + 1) * 128],
                        rhs=hT[:, kc, :],
                        start=(kc == 0), stop=(kc == KC - 1),
                    )
            for blk in range(KC):
                if blk % 2 == 0:
                    nc.vector.tensor_scalar(x1T[:, blk, :], px1[:, blk, :],
                                            b0_v[:, blk:blk + 1], 0.0,
                                            Alu.add, Alu.max)
                else:
                    nc.scalar.activation(x1T[:, blk, :], px1[:, blk, :],
                                         Act.Relu, bias=b0_v[:, blk:blk + 1])

            x2 = cpool.tile([128, KC, 128], f32, tag="x2")
            px2 = mpsum.tile([128, KC, 128], f32, tag="mm", name="px2")
            for blk in range(KC):
                for kc in range(KC):
                    nc.tensor.matmul(
                        px2[:, blk, :],
                        lhsT=w1T_v[:, kc, blk * 128:(blk + 1) * 128],
                        rhs=x1T[:, kc, :],
                        start=(kc == 0), stop=(kc == KC - 1),
                    )
            for blk in range(KC):
                if blk % 2 == 0:
                    nc.vector.tensor_scalar(x2[:, blk, :], px2[:, blk, :],
                                            b1_v[:, blk:blk + 1], 0.0,
                                            Alu.add, Alu.max)
                else:
                    nc.scalar.activation(x2[:, blk, :], px2[:, blk, :],
                                         Act.Relu, bias=b1_v[:, blk:blk + 1])

            nc.sync.dma_start(out_e, x2[:])

    nc.compile()
    return nc


def _get_nc():
    if "nc" not in _CACHE:
        _CACHE["nc"] = _build_nc()
    return _CACHE["nc"]


def kernel(gcn_inputs, word_seq_len, adj_matrix, dep_label_matrix,
           w_params, mlp_w0, mlp_b0, mlp_w1, mlp_b1, **_unused):
    from concourse.bass_utils import run_bass_kernel_spmd

    gcn = np.asarray(gcn_inputs, dtype=np.float32)
    adj = np.asarray(adj_matrix, dtype=np.float32)
    lab = np.asarray(dep_label_matrix)
    w = np.asarray(w_params, dtype=np.float32)
    w0 = np.asarray(mlp_w0, dtype=np.float32)
    w1 = np.asarray(mlp_w1, dtype=np.float32)
    b0 = np.asarray(mlp_b0, dtype=np.float32)
    b1 = np.asarray(mlp_b1, dtype=np.float32)

    # wT[kmod, l, kc, d] = w[l, d, kc*128+kmod]  (shared by all cores)
    wT = w.transpose(0, 2, 1).reshape(L, KC, 128, D).transpose(2, 0, 1, 3)
    wT = np.ascontiguousarray(wT).astype(np.float16)
    w0T = w0.T.reshape(KC, 128, D).transpose(1, 0, 2)
    w1T = w1.T.reshape(KC, 128, D).transpose(1, 0, 2)
    mlpw = np.ascontiguousarray(
        np.stack([w0T, w1T], axis=1)).astype(np.float16)   # [128, 2, KC, D]
    b0r = b0.reshape(KC, 128).T                            # [128, KC]
    b1r = b1.reshape(KC, 128).T
    labf = lab.astype(np.float32)

    in_maps = []
    for c in range(NCORES):
        packc = np.empty((N, P_TOT), dtype=np.float32)
        packc[:, P_GCN:P_GCN + D] = gcn[c]
        packc[:, P_ADJT:P_ADJT + N] = adj[c].T
        packc[:, P_LABT:P_LABT + N] = labf[c].T
        packc[:, P_ADJR:P_ADJR + N] = adj[c]
        packc[:, P_B0:P_B0 + KC] = b0r
        packc[:, P_B1:P_B1 + KC] = b1r
        in_maps.append({
            "pack": packc,
            "wT": wT,
            "mlpw": mlpw,
        })

    nc = _get_nc()
    res = run_bass_kernel_spmd(nc, in_maps, list(range(NCORES)))

    out = np.empty((B, N, D), dtype=np.float32)
    for c in range(NCORES):
        arr = res.results[c]["out"]          # [dmod, dblk, i]
        out[c] = np.transpose(arr, (2, 1, 0)).reshape(N, D)
    return out


# revision 13
# speedup vs baseline: 1.0660x; 1.0660x over previous
"""DepLabeledGCN Trainium2 kernel — data-parallel variant (no collectives).

Each core processes ITS OWN batch with ALL 48 label matrices:
    s-phase:  sT[kc,l] chunks = per-label masked-adjacency matmuls (fp16,
              masks exact 0/1), label PAIRS fused into N=256 matmuls
    msum:     msg = sum_{l,kc} sT[kc,l] @ W_l^T[kc], 192 accumulating
              matmuls into one PSUM bank per layer
    relu(msg * 1/denom) -> next layer h (4 kc-chunk DVE ops)
then the 2-layer MLP (PE-transpose + packed PSUM) on the same core.

v2 scheduling changes vs baseline:
  - all small inputs packed into ONE dram tensor / one DMA
  - masks for pair 0 + chunked h0 cast first => first matmul ~2.5us earlier
  - sT tile keeps the PSUM layout [q,kc,l,i]; psum->sbuf copy is two
    contiguous halves on different engines (vector/scalar/gpsimd)
  - weight DMAs alternate sync/gpsimd queues (2 DGE streams)
  - relu + h0 casts split into kc chunks to cut layer-boundary latency
  - 16 pair-granular wres loads, 1 MLP-weight load, 1 output DMA
    (fewer queues => shorter semaphore-reset teardown)
"""

import sys

if '/opt/trn_rl_repo' not in sys.path:
    sys.path.insert(0, '/opt/trn_rl_repo')

import numpy as np

B, N, D, L = 8, 128, 512, 48
NCORES = 8
KC = D // 128
NUM_LAYERS = 2
R_RES = 32              # labels kept resident for layer 2
NP = L // 2             # label pairs per layer

# pack column offsets (fp32 units); adjT/labT first so the mask DMA
# chunk can land before the gcn chunk
P_ADJT = 0
P_LABT = 128
P_GCN = 256
P_ADJR = 768
P_B0 = 896
P_B1 = 900
P_TOT = 904
P_SPLIT = 256           # DMA 1: [0, 256) masks;  DMA 2: [256, 904)

_CACHE = {}


def _build_nc():
    import concourse.bass as bass
    import concourse.mybir as mybir
    import concourse.tile as tile
    from concourse import bacc
    from concourse.masks import make_identity

    dt = mybir.dt
    f32 = dt.float32
    f16 = dt.float16
    Alu = mybir.AluOpType

    nc = bacc.Bacc("TRN2", target_bir_lowering=False, debug=False,
                   num_devices=NCORES)

    pack_e = nc.dram_tensor("pack", [N, P_TOT], f32, kind="ExternalInput").ap()
    wT_e = nc.dram_tensor("wT", [128, L, KC, D], f16, kind="ExternalInput").ap()
    mlpw_e = nc.dram_tensor("mlpw", [128, 2, KC, D], f16,
                            kind="ExternalInput").ap()
    out_e = nc.dram_tensor("out", [128, KC, 128], f32,
                           kind="ExternalOutput").ap()

    with tile.TileContext(nc) as tc:
        with (
            tc.tile_pool(name="const", bufs=1) as cpool,
            tc.tile_pool(name="sT", bufs=3) as sT_pool,
            tc.tile_pool(name="wst", bufs=4) as wst_pool,
            tc.tile_pool(name="spsum", bufs=3, space="PSUM") as spsum,
            tc.tile_pool(name="mpsum", bufs=2, space="PSUM") as mpsum,
        ):
            # -------- critical-path input load (two chunks) -----------------
            pack_sb = cpool.tile([128, P_TOT], f32, tag="pack")
            nc.sync.dma_start(pack_sb[:, :P_SPLIT], pack_e[:, :P_SPLIT])
            nc.sync.dma_start(pack_sb[:, P_SPLIT:], pack_e[:, P_SPLIT:])
            gcn_v = pack_sb[:, P_GCN:P_GCN + D]
            adjT_v = pack_sb[:, P_ADJT:P_ADJT + N]
            labT_v = pack_sb[:, P_LABT:P_LABT + N]
            adjR_v = pack_sb[:, P_ADJR:P_ADJR + N]
            b0_v = pack_sb[:, P_B0:P_B0 + KC]
            b1_v = pack_sb[:, P_B1:P_B1 + KC]

            # resident weights, pair-granular, single HW queue (per-core DMA
            # is ~410 GB/s aggregate; splitting queues only delays the
            # early pairs the msum is waiting on)
            wres = cpool.tile([128, R_RES, KC, D], f16, tag="wres")
            for p in range(R_RES // 2):
                nc.sync.dma_start(wres[:, 2 * p:2 * p + 2],
                                  wT_e[:, 2 * p:2 * p + 2])

            h = [cpool.tile([128, D], f16, tag=f"h{ly}", name=f"h{ly}")
                 for ly in range(NUM_LAYERS + 1)]

            # -------- masks: maskT[j, l, i] = (labT == l) * adjT ------------
            maskT = cpool.tile([128, L, N], f16, tag="maskT")

            def emit_mask(l):
                nc.vector.scalar_tensor_tensor(
                    out=maskT[:, l, :],
                    in0=labT_v,
                    scalar=float(l),
                    in1=adjT_v,
                    op0=Alu.is_equal,
                    op1=Alu.mult,
                )

            # pair-0 masks first, then h0 chunks (vector/scalar), then more
            emit_mask(0)
            emit_mask(1)
            for kc in range(KC):
                sl = slice(kc * 128, (kc + 1) * 128)
                if kc % 2 == 0:
                    nc.vector.tensor_copy(h[0][:, sl], gcn_v[:, sl])
                else:
                    nc.scalar.copy(h[0][:, sl], gcn_v[:, sl])
            for l in range(2, 12):
                emit_mask(l)

            den = cpool.tile([128, 1], f32, tag="den")
            nc.vector.tensor_reduce(den[:], adjR_v, mybir.AxisListType.X,
                                    Alu.add)
            nc.vector.tensor_scalar_add(den[:], den[:], 1.0)
            recip = cpool.tile([128, 1], f32, tag="recip")
            nc.vector.reciprocal(recip[:], den[:])

            # identity for the MLP transposes (gpsimd, idle at start)
            identity = cpool.tile([128, 128], f16, tag="ident")
            make_identity(nc, identity[:])

            # -------- GCN layers --------------------------------------------
            Act = mybir.ActivationFunctionType

            def emit_s(ly, p):
                """s-phase for label pair p: one N=256 matmul per kc.
                psum AND sbuf tile share layout [q, kc, l, i] so the
                psum->sbuf cast is contiguous; split into two halves on
                vector + scalar (gpsimd has no PSUM access)."""
                ps = spsum.tile([128, KC, 2, 128], f32, tag="spsum",
                                name="spsum")
                for kc in range(KC):
                    nc.tensor.matmul(
                        ps[:, kc, :, :],
                        lhsT=h[ly][:, kc * 128:(kc + 1) * 128],
                        rhs=maskT[:, 2 * p:2 * p + 2, :],
                        start=True, stop=True,
                    )
                sT = sT_pool.tile([128, KC, 2, 128], f16, tag="sT", name="sT")
                nc.vector.tensor_copy(sT[:, 0:2], ps[:, 0:2])
                nc.scalar.copy(sT[:, 2:4], ps[:, 2:4])
                return sT

            def get_w(ly, p):
                """Weight pair p: resident slice or streamed tile."""
                if 2 * p + 1 < R_RES:
                    return wres[:, 2 * p:2 * p + 2]
                w = wst_pool.tile([128, 2, KC, D], f16, tag="wst", name="wst")
                nc.sync.dma_start(w[:], wT_e[:, 2 * p:2 * p + 2])
                return w

            for ly in range(NUM_LAYERS):
                pm = mpsum.tile([128, D], f32, tag="mm", name="mm")
                sT_q = [emit_s(ly, 0), emit_s(ly, 1)]
                for p in range(NP):
                    if ly == 0 and 2 * (p + 6) < L:
                        emit_mask(2 * (p + 6))
                        emit_mask(2 * (p + 6) + 1)
                    if p + 2 < NP:
                        sT_q.append(emit_s(ly, p + 2))
                    w = get_w(ly, p)
                    sT = sT_q[p]
                    for kc in range(KC):
                        for l2 in range(2):
                            i = p * 2 * KC + kc * 2 + l2
                            nc.tensor.matmul(
                                pm[:],
                                lhsT=sT[:, kc, l2, :],
                                rhs=w[:, l2, kc, :],
                                start=(i == 0), stop=(i == L * KC - 1),
                            )
                if ly == 0:
                    # MLP weights: load during layer 2 (slack window)
                    mlpw_sb = cpool.tile([128, 2, KC, D], f16, tag="mlpw")
                    nc.sync.dma_start(mlpw_sb[:], mlpw_e)
                # relu(msg * recip) -> next h (fp16), chunked per kc
                for kc in range(KC):
                    sl = slice(kc * 128, (kc + 1) * 128)
                    if kc % 2 == 0:
                        nc.vector.tensor_scalar(h[ly + 1][:, sl], pm[:, sl],
                                                recip[:], 0.0,
                                                Alu.mult, Alu.max)
                    else:
                        nc.scalar.activation(h[ly + 1][:, sl], pm[:, sl],
                                             Act.Relu, scale=recip[:])

            # -------- MLP ---------------------------------------------------
            w0T_v = mlpw_sb[:, 0]
            w1T_v = mlpw_sb[:, 1]
            h_own = h[NUM_LAYERS]
            hT = cpool.tile([128, KC, 128], f16, tag="hT")
            pt = mpsum.tile([128, KC, 128], f16, tag="mm", name="ptr")
            for kc in range(KC):
                nc.tensor.transpose(pt[:, kc, :],
                                    h_own[:, kc * 128:(kc + 1) * 128],
                                    identity[:])
            nc.vector.tensor_copy(hT[:, 0:2], pt[:, 0:2])
            nc.scalar.copy(hT[:, 2:4], pt[:, 2:4])

            x1T = cpool.tile([128, KC, 128], f16, tag="x1T")
            px1 = mpsum.tile([128, KC, 128], f32, tag="mm", name="px1")
            for blk in range(KC):
                for kc in range(KC):
                    nc.tensor.matmul(
                        px1[:, blk, :],
                        lhsT=w0T_v[:, kc, blk * 128:(blk + 1) * 128],
                        rhs=hT[:, kc, :],
                        start=(kc == 0), stop=(kc == KC - 1),
                    )
            for blk in range(KC):
                if blk % 2 == 0:
                    nc.vector.tensor_scalar(x1T[:, blk, :], px1[:, blk, :],
                                            b0_v[:, blk:blk + 1], 0.0,
                                            Alu.add, Alu.max)
                else:
                    nc.scalar.activation(x1T[:, blk, :], px1[:, blk, :],
                                         Act.Relu, bias=b0_v[:, blk:blk + 1])

            x2 = cpool.tile([128, KC, 128], f32, tag="x2")
            px2 = mpsum.tile([128, KC, 128], f32, tag="mm", name="px2")
            for blk in range(KC):
                for kc in range(KC):
                    nc.tensor.matmul(
                        px2[:, blk, :],
                        lhsT=w1T_v[:, kc, blk * 128:(blk + 1) * 128],
                        rhs=x1T[:, kc, :],
                        start=(kc == 0), stop=(kc == KC - 1),
                    )
            for blk in range(KC):
                if blk % 2 == 0:
                    nc.vector.tensor_scalar(x2[:, blk, :], px2[:, blk, :],
                                            b1_v[:, blk:blk + 1], 0.0,
                                            Alu.add, Alu.max)
                else:
                    nc.scalar.activation(x2[:, blk, :], px2[:, blk, :],
                                         Act.Relu, bias=b1_v[:, blk:blk + 1])

            nc.sync.dma_start(out_e, x2[:])

    nc.compile()
    return nc


def _get_nc():
    if "nc" not in _CACHE:
        _CACHE["nc"] = _build_nc()
    return _CACHE["nc"]


def kernel(gcn_inputs, word_seq_len, adj_matrix, dep_label_matrix,
           w_params, mlp_w0, mlp_b0, mlp_w1, mlp_b1, **_unused):
    from concourse.bass_utils import run_bass_kernel_spmd

    gcn = np.asarray(gcn_inputs, dtype=np.float32)
    adj = np.asarray(adj_matrix, dtype=np.float32)
    lab = np.asarray(dep_label_matrix)
    w = np.asarray(w_params, dtype=np.float32)
    w0 = np.asarray(mlp_w0, dtype=np.float32)
    w1 = np.asarray(mlp_w1, dtype=np.float32)
    b0 = np.asarray(mlp_b0, dtype=np.float32)
    b1 = np.asarray(mlp_b1, dtype=np.float32)

    # wT[kmod, l, kc, d] = w[l, d, kc*128+kmod]  (shared by all cores)
    wT = w.transpose(0, 2, 1).reshape(L, KC, 128, D).transpose(2, 0, 1, 3)
    wT = np.ascontiguousarray(wT).astype(np.float16)
    w0T = w0.T.reshape(KC, 128, D).transpose(1, 0, 2)
    w1T = w1.T.reshape(KC, 128, D).transpose(1, 0, 2)
    mlpw = np.ascontiguousarray(
        np.stack([w0T, w1T], axis=1)).astype(np.float16)   # [128, 2, KC, D]
    b0r = b0.reshape(KC, 128).T                            # [128, KC]
    b1r = b1.reshape(KC, 128).T
    labf = lab.astype(np.float32)

    in_maps = []
    for c in range(NCORES):
        packc = np.empty((N, P_TOT), dtype=np.float32)
        packc[:, P_GCN:P_GCN + D] = gcn[c]
        packc[:, P_ADJT:P_ADJT + N] = adj[c].T
        packc[:, P_LABT:P_LABT + N] = labf[c].T
        packc[:, P_ADJR:P_ADJR + N] = adj[c]
        packc[:, P_B0:P_B0 + KC] = b0r
        packc[:, P_B1:P_B1 + KC] = b1r
        in_maps.append({
            "pack": packc,
            "wT": wT,
            "mlpw": mlpw,
        })

    nc = _get_nc()
    res = run_bass_kernel_spmd(nc, in_maps, list(range(NCORES)))

    out = np.empty((B, N, D), dtype=np.float32)
    for c in range(NCORES):
        arr = res.results[c]["out"]          # [dmod, dblk, i]
        out[c] = np.transpose(arr, (2, 1, 0)).reshape(N, D)
    return out


# revision 14
# speedup vs baseline: 1.1432x; 1.0725x over previous
"""DepLabeledGCN Trainium2 kernel — data-parallel variant (no collectives).

Each core processes ITS OWN batch with ALL 48 label matrices:
    s-phase:  sT[kc,l] chunks = per-label masked-adjacency matmuls (fp16,
              masks exact 0/1), label PAIRS fused into N=256 matmuls
    msum:     msg = sum_{l,kc} sT[kc,l] @ W_l^T[kc], 192 accumulating
              matmuls into one PSUM bank per layer
    relu(msg * 1/denom) -> next layer h (chunked DVE/Act ops)
then the 2-layer MLP (PE-transpose + packed PSUM) on the same core.

Weights: 24 MB fp16 streamed per label from HBM on ONE hw queue (per-core
DMA is ~410 GB/s aggregate; more queues only delays the early pairs).
The first R_RES labels stay SBUF-resident for layer 2.

Scheduling details (measured on hw traces):
  - sT tile keeps the PSUM layout [q,kc,l,i]; the psum->sbuf cast is two
    contiguous halves on vector + scalar concurrently (gpsimd cannot
    access PSUM).  msum runs l2-major so each matmul only depends on
    one label's weight DMA (layer 1 is DMA-starved; finer deps matter).
  - weight DMAs stay per-label for the same reason.
  - h0 cast and the layer-boundary relu are chunked per kc to shorten
    the critical path into each layer's first matmuls.
"""

import sys

if '/opt/trn_rl_repo' not in sys.path:
    sys.path.insert(0, '/opt/trn_rl_repo')

import numpy as np

B, N, D, L = 8, 128, 512, 48
NCORES = 8
KC = D // 128
NUM_LAYERS = 2
R_RES = 32              # labels kept resident for layer 2
NP = L // 2             # label pairs per layer

_CACHE = {}


def _build_nc():
    import concourse.bass as bass
    import concourse.mybir as mybir
    import concourse.tile as tile
    from concourse import bacc
    from concourse.masks import make_identity

    dt = mybir.dt
    f32 = dt.float32
    f16 = dt.float16
    Alu = mybir.AluOpType
    Act = mybir.ActivationFunctionType

    nc = bacc.Bacc("TRN2", target_bir_lowering=False, debug=False,
                   num_devices=NCORES)

    gcn_e = nc.dram_tensor("gcn", [N, D], f32, kind="ExternalInput").ap()
    adjT_e = nc.dram_tensor("adjT", [N, N], f32, kind="ExternalInput").ap()
    labT_e = nc.dram_tensor("labT", [N, N], f32, kind="ExternalInput").ap()
    # misc: adjR (row-major adj) + b0 + b1 packed
    misc_e = nc.dram_tensor("misc", [N, N + 2 * KC], f32,
                            kind="ExternalInput").ap()
    wT_e = nc.dram_tensor("wT", [128, L, KC, D], f16, kind="ExternalInput").ap()
    mlpw_e = nc.dram_tensor("mlpw", [128, 2, KC, D], f16,
                            kind="ExternalInput").ap()
    out_e = nc.dram_tensor("out", [128, KC, 128], f32,
                           kind="ExternalOutput").ap()

    with tile.TileContext(nc) as tc:
        with (
            tc.tile_pool(name="const", bufs=1) as cpool,
            tc.tile_pool(name="sT", bufs=3) as sT_pool,
            tc.tile_pool(name="wst", bufs=5) as wst_pool,
            tc.tile_pool(name="spsum", bufs=3, space="PSUM") as spsum,
            tc.tile_pool(name="mpsum", bufs=2, space="PSUM") as mpsum,
        ):
            # -------- critical-path input loads -----------------------------
            adjT_sb = cpool.tile([128, N], f32, tag="adjT")
            nc.sync.dma_start(adjT_sb[:], adjT_e)
            labT_sb = cpool.tile([128, N], f32, tag="labT")
            nc.sync.dma_start(labT_sb[:], labT_e)
            gcn_sb = cpool.tile([128, D], f32, tag="gcn_sb")
            nc.sync.dma_start(gcn_sb[:], gcn_e)

            h = [cpool.tile([128, D], f16, tag=f"h{ly}", name=f"h{ly}")
                 for ly in range(NUM_LAYERS + 1)]
            # h0 cast chunked on scalar (vector is busy with masks)
            for kc in range(KC):
                sl = slice(kc * 128, (kc + 1) * 128)
                nc.scalar.copy(h[0][:, sl], gcn_sb[:, sl])

            # resident weights, loaded per label (just-in-time for layer 1)
            wres = cpool.tile([128, R_RES, KC, D], f16, tag="wres")
            for l in range(R_RES):
                nc.sync.dma_start(wres[:, l], wT_e[:, l])

            # -------- masks: maskT[j, l, i] = (labT == l) * adjT ------------
            # pairs 0..5 upfront; the rest interleaved into the layer-1 loop
            maskT = cpool.tile([128, L, N], f16, tag="maskT")

            def emit_mask(l):
                nc.vector.scalar_tensor_tensor(
                    out=maskT[:, l, :],
                    in0=labT_sb[:],
                    scalar=float(l),
                    in1=adjT_sb[:],
                    op0=Alu.is_equal,
                    op1=Alu.mult,
                )

            for l in range(12):
                emit_mask(l)

            misc_sb = cpool.tile([128, N + 2 * KC], f32, tag="misc")
            nc.sync.dma_start(misc_sb[:], misc_e)
            adjR_v = misc_sb[:, 0:N]
            b0_v = misc_sb[:, N:N + KC]
            b1_v = misc_sb[:, N + KC:N + 2 * KC]

            den = cpool.tile([128, 1], f32, tag="den")
            nc.vector.tensor_reduce(den[:], adjR_v, mybir.AxisListType.X,
                                    Alu.add)
            nc.vector.tensor_scalar_add(den[:], den[:], 1.0)
            recip = cpool.tile([128, 1], f32, tag="recip")
            nc.vector.reciprocal(recip[:], den[:])

            # identity for the MLP transposes (gpsimd, idle at start)
            identity = cpool.tile([128, 128], f16, tag="ident")
            make_identity(nc, identity[:])

            # -------- GCN layers --------------------------------------------
            def emit_s(ly, p):
                """s-phase for label pair p: one N=256 matmul per kc.
                psum AND sbuf tiles share layout [q, kc, l, i] so the
                psum->sbuf cast is contiguous; two halves on vector+scalar."""
                ps = spsum.tile([128, KC, 2, 128], f32, tag="spsum",
                                name="spsum")
                for kc in range(KC):
                    nc.tensor.matmul(
                        ps[:, kc, :, :],
                        lhsT=h[ly][:, kc * 128:(kc + 1) * 128],
                        rhs=maskT[:, 2 * p:2 * p + 2, :],
                        start=True, stop=True,
                    )
                sT = sT_pool.tile([128, KC, 2, 128], f16, tag="sT", name="sT")
                nc.vector.tensor_copy(sT[:, 0:2], ps[:, 0:2])
                nc.scalar.copy(sT[:, 2:4], ps[:, 2:4])
                return sT

            def get_w(ly, p):
                """Weight pair p: resident slice or streamed tile
                (per-label DMAs keep the msum deps fine-grained)."""
                if 2 * p + 1 < R_RES:
                    return wres[:, 2 * p:2 * p + 2]
                w = wst_pool.tile([128, 2, KC, D], f16, tag="wst", name="wst")
                nc.sync.dma_start(w[:, 0], wT_e[:, 2 * p])
                nc.sync.dma_start(w[:, 1], wT_e[:, 2 * p + 1])
                return w

            for ly in range(NUM_LAYERS):
                pm = mpsum.tile([128, D], f32, tag="mm", name="mm")
                sT_q = [emit_s(ly, 0), emit_s(ly, 1)]
                for p in range(NP):
                    if ly == 0 and 2 * (p + 6) < L:
                        emit_mask(2 * (p + 6))
                        emit_mask(2 * (p + 6) + 1)
                    if p + 2 < NP:
                        sT_q.append(emit_s(ly, p + 2))
                    w = get_w(ly, p)
                    sT = sT_q[p]
                    for l2 in range(2):
                        for kc in range(KC):
                            i = (p * 2 + l2) * KC + kc
                            nc.tensor.matmul(
                                pm[:],
                                lhsT=sT[:, kc, l2, :],
                                rhs=w[:, l2, kc, :],
                                start=(i == 0), stop=(i == L * KC - 1),
                            )
                if ly == 0:
                    # MLP weights: load during layer 2 (slack window)
                    mlpw_sb = cpool.tile([128, 2, KC, D], f16, tag="mlpw")
                    nc.sync.dma_start(mlpw_sb[:], mlpw_e)
                # relu(msg * recip) -> next h (fp16), chunked per kc
                for kc in range(KC):
                    sl = slice(kc * 128, (kc + 1) * 128)
                    if kc % 2 == 0:
                        nc.vector.tensor_scalar(h[ly + 1][:, sl], pm[:, sl],
                                                recip[:], 0.0,
                                                Alu.mult, Alu.max)
                    else:
                        nc.scalar.activation(h[ly + 1][:, sl], pm[:, sl],
                                             Act.Relu, scale=recip[:])

            # -------- MLP ---------------------------------------------------
            w0T_v = mlpw_sb[:, 0]
            w1T_v = mlpw_sb[:, 1]
            h_own = h[NUM_LAYERS]
            hT = cpool.tile([128, KC, 128], f16, tag="hT")
            pt = mpsum.tile([128, KC, 128], f16, tag="mm", name="ptr")
            for kc in range(KC):
                nc.tensor.transpose(pt[:, kc, :],
                                    h_own[:, kc * 128:(kc + 1) * 128],
                                    identity[:])
            nc.vector.tensor_copy(hT[:, 0:2], pt[:, 0:2])
            nc.scalar.copy(hT[:, 2:4], pt[:, 2:4])

            x1T = cpool.tile([128, KC, 128], f16, tag="x1T")
            px1 = mpsum.tile([128, KC, 128], f32, tag="mm", name="px1")
            for blk in range(KC):
                for kc in range(KC):
                    nc.tensor.matmul(
                        px1[:, blk, :],
                        lhsT=w0T_v[:, kc, blk * 128:(blk + 1) * 128],
                        rhs=hT[:, kc, :],
                        start=(kc == 0), stop=(kc == KC - 1),
                    )
            for blk in range(KC):
                if blk % 2 == 0:
                    nc.vector.tensor_scalar(x1T[:, blk, :], px1[:, blk, :],
                                            b0_v[:, blk:blk + 1], 0.0,
                                            Alu.add, Alu.max)
                else:
                    nc.scalar.activation(x1T[:, blk, :], px1[:, blk, :],
                                         Act.Relu, bias=b0_v[:, blk:blk + 1])

            x2 = cpool.tile([128, KC, 128], f32, tag="x2")
            px2 = mpsum.tile([128, KC, 128], f32, tag="mm", name="px2")
            for blk in range(KC):
                for kc in range(KC):
                    nc.tensor.matmul(
                        px2[:, blk, :],
                        lhsT=w1T_v[:, kc, blk * 128:(blk + 1) * 128],
                        rhs=x1T[:, kc, :],
                        start=(kc == 0), stop=(kc == KC - 1),
                    )
            for blk in range(KC):
                if blk % 2 == 0:
                    nc.vector.tensor_scalar(x2[:, blk, :], px2[:, blk, :],
                                            b1_v[:, blk:blk + 1], 0.0,
                                            Alu.add, Alu.max)
                else:
                    nc.scalar.activation(x2[:, blk, :], px2[:, blk, :],
                                         Act.Relu, bias=b1_v[:, blk:blk + 1])

            nc.sync.dma_start(out_e, x2[:])

    nc.compile()
    return nc


def _get_nc():
    if "nc" not in _CACHE:
        _CACHE["nc"] = _build_nc()
    return _CACHE["nc"]


def kernel(gcn_inputs, word_seq_len, adj_matrix, dep_label_matrix,
           w_params, mlp_w0, mlp_b0, mlp_w1, mlp_b1, **_unused):
    from concourse.bass_utils import run_bass_kernel_spmd

    gcn = np.asarray(gcn_inputs, dtype=np.float32)
    adj = np.asarray(adj_matrix, dtype=np.float32)
    lab = np.asarray(dep_label_matrix)
    w = np.asarray(w_params, dtype=np.float32)
    w0 = np.asarray(mlp_w0, dtype=np.float32)
    w1 = np.asarray(mlp_w1, dtype=np.float32)
    b0 = np.asarray(mlp_b0, dtype=np.float32)
    b1 = np.asarray(mlp_b1, dtype=np.float32)

    # wT[kmod, l, kc, d] = w[l, d, kc*128+kmod]  (shared by all cores)
    wT = w.transpose(0, 2, 1).reshape(L, KC, 128, D).transpose(2, 0, 1, 3)
    wT = np.ascontiguousarray(wT).astype(np.float16)
    w0T = w0.T.reshape(KC, 128, D).transpose(1, 0, 2)
    w1T = w1.T.reshape(KC, 128, D).transpose(1, 0, 2)
    mlpw = np.ascontiguousarray(
        np.stack([w0T, w1T], axis=1)).astype(np.float16)   # [128, 2, KC, D]
    b0r = b0.reshape(KC, 128).T                            # [128, KC]
    b1r = b1.reshape(KC, 128).T
    labf = lab.astype(np.float32)

    in_maps = []
    for c in range(NCORES):
        miscc = np.empty((N, N + 2 * KC), dtype=np.float32)
        miscc[:, 0:N] = adj[c]
        miscc[:, N:N + KC] = b0r
        miscc[:, N + KC:N + 2 * KC] = b1r
        in_maps.append({
            "gcn": gcn[c],
            "adjT": np.ascontiguousarray(adj[c].T),
            "labT": np.ascontiguousarray(labf[c].T),
            "misc": miscc,
            "wT": wT,
            "mlpw": mlpw,
        })

    nc = _get_nc()
    res = run_bass_kernel_spmd(nc, in_maps, list(range(NCORES)))

    out = np.empty((B, N, D), dtype=np.float32)
    for c in range(NCORES):
        arr = res.results[c]["out"]          # [dmod, dblk, i]
        out[c] = np.transpose(arr, (2, 1, 0)).reshape(N, D)
    return out
